# revision 102
# baseline (speedup 1.0000x reference)
"""Trainium2 Bass kernel for nn_Attention_12429635355261 (sparse_attention).

Data-parallel over batch: 32 batch items -> 8 NeuronCores x 4.
Per core, batch items are processed in 2 pair-groups of 2 (free dims pack
the pair side-by-side so matmul moving free = 512).

All layouts keep the matmul contraction on partitions:
  spatial:  qT/kT [(h d), t];  dotsT/exp/attn in [j, i] layout;
            conv via grouped (c, j'-band) partition tiles + banded stationaries;
            attn@v -> OF [(h d), i]; proj -> outputT [c, t]
  spectral: qsT/ksT [u, c]; dots_s [c, c'] natural; conv via 64-row halo bands
  final:    result [t, e] = outputT.T @ attn_s_conv
"""
import os
import numpy as np

import concourse.bass as bass
import concourse.mybir as mybir
from concourse import bacc, tile
from concourse.tile import add_dep_helper
from concourse.bass_utils import run_bass_kernel_spmd

FP32 = mybir.dt.float32
FP32R = mybir.dt.float32r
BF16 = mybir.dt.bfloat16

H, D = 8, 64
T = 256
C = 512
SCALE = D ** -0.5
NCORES = 8
NB = 4            # batch per core
NPAIR = 2         # pair-groups per core

CONV_OUT = 14
CONV_IN = 16
NBANDS = (T + CONV_OUT - 1) // CONV_OUT   # 19

SB_OUT = 64
SB_IN = 66
NSB = C // SB_OUT                          # 8

_cache = {}


def _to_bf16(a):
    import ml_dtypes
    return np.asarray(a, np.float32).astype(ml_dtypes.bfloat16)


def build_conv_stationaries(w_sconv):
    LH = np.zeros((3, 128, 112), np.float32)
    for ki in range(3):
        for c in range(8):
            for rel in range(16):
                for o in range(8):
                    for jrel in range(14):
                        kj = rel - jrel
                        if 0 <= kj <= 2:
                            LH[ki, rel * 8 + c, jrel * 8 + o] = \
                                w_sconv[o, c, ki, kj]
    return LH


def build_spec_stationaries(w3):
    B = np.zeros((3, SB_IN, SB_OUT), np.float32)
    for de in range(3):
        for r in range(SB_IN):
            lo = max(0, r - 2)
            hi = min(SB_OUT - 1, r)
            for m in range(lo, hi + 1):
                dc = r - m
                if 0 <= dc <= 2:
                    B[de, r, m] = w3[dc, de]
    return B


def _ones8():
    s = np.zeros((128, 8, 8), np.float32)
    for h in range(8):
        s[:, h, h] = 1.0
    return s


def _sel16():
    s = np.zeros((8, 128), np.float32)
    for c in range(8):
        for rel in range(16):
            s[c, rel * 8 + c] = 1.0
    return s


def build_program(has_sconv_bias):
    nc = bacc.Bacc("TRN2", target_bir_lowering=False, debug=False)

    def din(name, shape, dt=FP32):
        return nc.dram_tensor(name, list(shape), dt, kind="ExternalInput").ap()

    x_d = din("x", (NB, 128, 2, C), BF16)   # host pre-swizzled (p, tt, c)
    w_qkv_d = din("w_qkv", (C, 3 * H * D), BF16)
    w_out_d = din("w_out_bf", (C, C), BF16)
    b_out_d = din("b_out_col", (128, 4))
    w_spq_d = din("w_spec_q", (T, T), BF16)
    w_spk_d = din("w_spec_k", (T, T), BF16)
    conv_stat_d = din("conv_stat", (128, 3, 112), BF16)
    spec_stat_d = din("spec_stat", (SB_IN, 3, SB_OUT), BF16)
    bias_row_d = din("bias_row", (1, 112), BF16)       # b_sconv expanded
    bsp_col_d = din("bsp_col", (128, 1))               # b_specconv bcast
    ones8_d = din("ones8", (128, 8, 8), BF16)          # denominator selectors
    sel16_d = din("sel16", (8, 128), BF16)             # rel-broadcast selector
    onesN_d = din("onesN", (1, C), BF16)               # rhs for bias rank-1
    ident_d = din("ident", (128, 128), BF16)           # PE transpose identity
    zeros_d = din("zeros16", (128, 2 * T), BF16)       # pad rows source
    out_d = nc.dram_tensor("out", [NB, T, C], FP32, kind="ExternalOutput").ap()

    r32 = lambda ap: ap.bitcast(FP32R)

    from contextlib import ExitStack
    with ExitStack() as stk:
        tc = stk.enter_context(tile.TileContext(nc))
        pool = lambda name, bufs, **kw: stk.enter_context(
            tc.tile_pool(name=name, bufs=bufs, **kw))
        wres = pool("wres", 1)
        xin = pool("xin", 2)
        xtp = pool("xtp", 2)
        qkp = pool("qkp", 2)
        vp = pool("vp", 2)
        ep = pool("ep", 2)
        rp = pool("rp", 2)
        gp = pool("gp", 6)
        cgp = pool("cgp", 6)
        acp = pool("acp", 2)
        spqk = pool("spqk", 2)
        asp = pool("asp", 1)
        hsp = pool("hsp", 1)
        ascp = pool("ascp", 1)
        resp = pool("resp", 2)
        ps = pool("ps", 7, space="PSUM")
        pss = pool("pss", 1, space="PSUM")
        if True:
            # ---- preload x for both pair-groups (ahead of weights; each
            # tile is split in half across the three DMA-capable queues --
            # one queue moves only ~20 GB/s, a full tile is 256 KB) ----
            x_all = []
            x_engs = [nc.gpsimd, nc.sync, nc.scalar]

            def load_x(pg, xq0):
                xp = []
                xq = xq0
                for bb in range(2):
                    t_ = xin.tile([128, 2, C], BF16, tag=f"x{bb}",
                                  name=f"x_{pg}_{bb}")
                    src = x_d[2 * pg + bb]
                    for half in range(2):
                        x_engs[xq % 3].dma_start(
                            t_[64 * half:64 * (half + 1), :, :],
                            src[64 * half:64 * (half + 1)])
                        xq += 1
                    xp.append(t_)
                x_all.append(xp)

            # ident first (transposes gate on it); then pg0's x
            ident = wres.tile([128, 128], BF16, tag="ident")
            nc.sync.dma_start(ident[:], ident_d[:])
            load_x(0, 0)
            w_spq_sb, w_spk_sb = [], []
            for tt in range(2):
                t_ = wres.tile([128, T], BF16, tag=f"wspq{tt}")
                nc.sync.dma_start(t_[:], w_spq_d[tt * 128:(tt + 1) * 128, :])
                w_spq_sb.append(t_)
                t_ = wres.tile([128, T], BF16, tag=f"wspk{tt}")
                nc.scalar.dma_start(t_[:], w_spk_d[tt * 128:(tt + 1) * 128, :])
                w_spk_sb.append(t_)
            w_qkv_sb = []
            wq_engs = (nc.sync, nc.scalar, nc.gpsimd)
            MQ = 3 * H * D // 2
            for ct in range(4):
                t_ = wres.tile([128, 3 * H * D], BF16, tag=f"wqkv{ct}")
                for half in range(2):
                    wq_engs[(2 * ct + half) % 3].dma_start(
                        t_[:, MQ * half:MQ * (half + 1)],
                        w_qkv_d[ct * 128:(ct + 1) * 128,
                                MQ * half:MQ * (half + 1)])
                w_qkv_sb.append(t_)
            load_x(1, 1)
            zeros16 = wres.tile([128, 2 * T], BF16, tag="zeros16")
            nc.sync.dma_start(zeros16[:], zeros_d[:])
            ones8 = wres.tile([128, 8, 8], BF16, tag="ones8")
            nc.sync.dma_start(ones8[:], ones8_d[:])
            w_out_sb = []
            for kt in range(4):
                t_ = wres.tile([128, C], BF16, tag=f"wout{kt}")
                (nc.sync if kt % 2 == 0 else nc.scalar).dma_start(
                    t_[:], w_out_d[kt * 128:(kt + 1) * 128, :])
                w_out_sb.append(t_)
            conv_stat = wres.tile([128, 3, 112], BF16, tag="cstat")
            nc.sync.dma_start(conv_stat[:], conv_stat_d[:])
            spec_stat = wres.tile([SB_IN, 3, SB_OUT], BF16, tag="sstat")
            nc.scalar.dma_start(spec_stat[:], spec_stat_d[:])
            b_out_col = wres.tile([128, 4], FP32, tag="bout")
            nc.sync.dma_start(b_out_col[:], b_out_d[:])
            bias_row = wres.tile([1, 112], BF16, tag="brow")
            nc.scalar.dma_start(bias_row[:], bias_row_d[:])
            bsp_col = wres.tile([128, 1], FP32, tag="bsp")
            nc.sync.dma_start(bsp_col[:], bsp_col_d[:])
            sel16 = wres.tile([8, 128], BF16, tag="sel16")
            nc.scalar.dma_start(sel16[:], sel16_d[:])
            onesN = wres.tile([1, C], BF16, tag="onesN")
            nc.sync.dma_start(onesN[:], onesN_d[:])

            # ============ xT via PE transpose ============
            # xT2[ct] [128, (bb, t)=512] f32
            def emit_transposes(pg):
                x_t = x_all[pg]
                xT2 = []
                for ct in range(4):
                    dst = xtp.tile([128, 2 * T], BF16, tag=f"xT{ct}",
                                   name=f"xT{ct}_{pg}")
                    for bb in range(2):
                        pt = ps.tile([128, T], FP32, tag="pb", name="pt")
                        for tt in range(2):
                            nc.tensor.matmul(
                                pt[:, tt * 128:(tt + 1) * 128],
                                x_t[bb][:, tt, ct * 128:(ct + 1) * 128],
                                ident[:], start=True, stop=True)
                        nc.vector.tensor_copy(dst[:, bb * T:(bb + 1) * T], pt[:])
                    xT2.append(dst)
                return xT2

            xT2_cache = {}
            for pg in range(NPAIR):
                x_t = x_all[pg]
                xT2 = xT2_cache.pop(pg, None)
                if xT2 is None:
                    xT2 = emit_transposes(pg)

                # ============ spectral qsT/ksT [u, c] bf16 ============
                # emitted before the conv so the conv gathers (scalar/sync
                # queues) execute while the PE runs the spectral matmuls
                sq = [[None] * 2 for _ in range(2)]   # [bb][ut] q
                sk = [[None] * 2 for _ in range(2)]
                for bb in range(2):
                    for ut in range(2):
                        for which, wsb, store in ((0, w_spq_sb, sq), (1, w_spk_sb, sk)):
                            pq = ps.tile([128, C], FP32, tag="pb", name="psq")
                            for tt in range(2):
                                nc.tensor.matmul(
                                    pq[:],
                                    wsb[tt][:, ut * 128:(ut + 1) * 128],
                                    x_t[bb][:, tt, :],
                                    start=(tt == 0), stop=(tt == 1))
                            t_ = spqk.tile([128, C], BF16, tag=f"sp{which}{bb}{ut}")
                            if (bb + ut) % 2 == 0:
                                nc.vector.tensor_copy(t_[:], pq[:])
                            else:
                                nc.scalar.copy(t_[:], pq[:])
                            store[bb][ut] = t_

                # ============ dots_s + softmax -> attn_s[bb] ============
                # attn_s[bb] tensor [128, (ct4, e)] bf16
                attn_s = []
                for bb in range(2):
                    asb = asp.tile([128, 4, C], BF16, tag=f"as{bb}")
                    ssum = rp.tile([128, 4], FP32, tag=f"ss{bb}")
                    for ct4 in range(4):
                        pd = ps.tile([128, C], FP32, tag="pb", name="pds")
                        for ut in range(2):
                            nc.tensor.matmul(
                                pd[:],
                                sq[bb][ut][:, ct4 * 128:(ct4 + 1) * 128],
                                sk[bb][ut][:], start=(ut == 0), stop=(ut == 1))
                        nc.scalar.activation(
                            asb[:, ct4, :], pd[:],
                            mybir.ActivationFunctionType.Exp, scale=SCALE,
                            accum_out=ssum[:, ct4:ct4 + 1])
                    rr = rp.tile([128, 4], FP32, tag=f"rs{bb}")
                    nc.vector.reciprocal(rr[:], ssum[:])
                    for ct4 in range(4):
                        if ct4 % 2 == 0:
                            nc.scalar.activation(
                                asb[:, ct4, :], asb[:, ct4, :],
                                mybir.ActivationFunctionType.Identity,
                                scale=rr[:, ct4:ct4 + 1])
                        else:
                            nc.vector.tensor_scalar(
                                asb[:, ct4, :], asb[:, ct4, :],
                                rr[:, ct4:ct4 + 1], None,
                                mybir.AluOpType.mult)
                    attn_s.append(asb)

                def gather_hs(bb, q):
                    c0 = SB_OUT * q
                    HS = hsp.tile([SB_IN, C], BF16, tag=f"HS{bb}{q}",
                                  name=f"HS{bb}{q}_{pg}")
                    # halo rows c' in [c0-1, c0+65)
                    lo = c0 - 1
                    src_lo = max(lo, 0)
                    src_hi = min(lo + SB_IN, C)
                    dma_eng = (nc.sync, nc.gpsimd, nc.scalar)[q % 3]
                    seg = src_lo
                    while seg < src_hi:
                        ct_ = seg // 128
                        seg_hi = min(src_hi, (ct_ + 1) * 128)
                        dma_eng.dma_start(
                            HS[seg - lo: seg_hi - lo, :],
                            attn_s[bb][seg - ct_ * 128: seg_hi - ct_ * 128,
                                       ct_, :])
                        seg = seg_hi
                    if q == 0:
                        dma_eng.dma_start(HS[0:1, :], zeros16[0:1, :C])
                    if lo + SB_IN > C:
                        dma_eng.dma_start(HS[SB_IN - 1:SB_IN, :],
                                          zeros16[0:1, :C])
                    return HS

                HSq = {(bb, q): gather_hs(bb, q)
                       for bb in range(2) for q in range(8)}

                # ============ qkT: [(h d), (b, t)] bf16, 8 M-tiles ============
                qkT = []
                for mt in range(8):
                    pq = ps.tile([128, 2 * T], FP32, tag="pb")
                    for ct in range(4):
                        nc.tensor.matmul(
                            pq[:], w_qkv_sb[ct][:, mt * 128:(mt + 1) * 128],
                            xT2[ct][:], start=(ct == 0), stop=(ct == 3))
                    t_ = qkp.tile([128, 2 * T], BF16, tag=f"qk{mt}")
                    if mt % 2 == 0:
                        nc.vector.tensor_copy(t_[:], pq[:])
                    else:
                        nc.scalar.copy(t_[:], pq[:])
                    qkT.append(t_)

                # ============ v: [t, (h d)] bf16 per (bb, tt) ============
                v_sb = [[None] * 2 for _ in range(2)]
                for bb in range(2):
                    for tt in range(2):
                        pv = ps.tile([128, C], FP32, tag="pb")
                        for ct in range(4):
                            nc.tensor.matmul(
                                pv[:],
                                xT2[ct][:, bb * T + tt * 128: bb * T + (tt + 1) * 128],
                                w_qkv_sb[ct][:, 2 * H * D:],
                                start=(ct == 0), stop=(ct == 3))
                        t_ = vp.tile([128, C], BF16, tag=f"v{bb}{tt}")
                        if (bb + tt) % 2 == 0:
                            nc.vector.tensor_copy(t_[:], pv[:])
                        else:
                            nc.scalar.copy(t_[:], pv[:])
                        v_sb[bb][tt] = t_

                # ============ dotsT + exp -> E [128, (h, jt, b, i)] bf16 ====
                E = ep.tile([128, H, 2, 2, T], BF16, tag="E")
                # jt-outer: all jt=0 exps land first so the pre-issued conv
                # gathers for bands 0..8 (jt=0 rows only) start mid-dots
                for jt in range(2):
                    for h in range(H):
                        hp = 64 * (h % 2)
                        pd = ps.tile([128, 2 * T], FP32, tag="pb")
                        for bb in range(2):
                            nc.tensor.matmul(
                                pd[:, bb * T:(bb + 1) * T],
                                qkT[4 + h // 2][hp:hp + 64,
                                                bb * T + jt * 128: bb * T + (jt + 1) * 128],
                                qkT[h // 2][hp:hp + 64, bb * T:(bb + 1) * T],
                                start=True, stop=True)
                        # exp(scale * dots) -> E block  [128, (b, i)]
                        nc.scalar.activation(
                            E[:, h, jt, :, :], pd[:],
                            mybir.ActivationFunctionType.Exp, scale=SCALE)

                # ============ softmax denominators (batched) ============
                # sp16[h] = sum_j E[j, h, :, :, :] ; rr16 = 1/sp16 ;
                # Rg[rel*8+c, :] = rr16[c, :]  (grouped broadcast for G scale)
                sp16 = pss.tile([8, 2 * T], FP32, tag="s")
                for h in range(H):
                    for jt in range(2):
                        nc.tensor.matmul(
                            sp16[:], ones8[:, h, :], E[:, h, jt, :, :],
                            start=(h == 0 and jt == 0),
                            stop=(h == H - 1 and jt == 1))
                rr16 = rp.tile([8, 2 * T], BF16, tag="rr")
                with nc.allow_low_precision(reason="bf16 recip"):
                    nc.vector.reciprocal(rr16[:], sp16[:])
                Rg_ps = ps.tile([128, 2 * T], FP32, tag="pb", name="Rg_ps")
                nc.tensor.matmul(Rg_ps[:], sel16[:], rr16[:],
                                 start=True, stop=True)
                Rg = rp.tile([128, 2 * T], BF16, tag="Rg")
                nc.vector.tensor_copy(Rg[:], Rg_ps[:])

                # ---- conv gather stage (reads RAW E -- independent of the
                # softmax denominators; pre-issued so transfers overlap the
                # spectral matmuls below) ----
                engs = (nc.scalar, nc.sync, nc.gpsimd)

                def gather_band(t):
                    j0 = CONV_OUT * t
                    G = gp.tile([128, 2 * T], BF16, tag="G", name=f"G{t}_{pg}")
                    lo = j0 - 1
                    src_lo = max(lo, 0)
                    src_hi = min(lo + CONV_IN, T)
                    gq = 2 * t
                    cuts = sorted({src_lo, src_hi} |
                                  {c for c in (lo + 8, 128)
                                   if src_lo < c < src_hi})
                    for seg, seg_hi in zip(cuts[:-1], cuts[1:]):
                        jt_ = seg // 128
                        p0 = seg - lo
                        engs[gq % 3].dma_start(
                            G[8 * p0: 8 * (p0 + seg_hi - seg), :],
                            E[seg - jt_ * 128: seg_hi - jt_ * 128, :, jt_, :, :])
                        gq += 1
                    if t == 0:
                        engs[gq % 3].dma_start(G[0:8, :], zeros16[0:8, :])
                        gq += 1
                    if j0 - 1 + CONV_IN > T:
                        npad = j0 - 1 + CONV_IN - T
                        engs[gq % 3].dma_start(G[8 * (CONV_IN - npad):128, :],
                                               zeros16[:8 * npad, :])
                    return G

                PRE = 5
                Gq = [gather_band(t) for t in range(PRE)]

                # ============ spatial conv (grouped bands) ============
                # AC[jt] [128, (h, b, i)] bf16
                AC = [acp.tile([128, H, 2, T], BF16, tag=f"AC{jt}", name=f"AC{jt}_{pg}")
                      for jt in range(2)]
                for t in range(NBANDS):
                    j0 = CONV_OUT * t
                    nj = min(CONV_OUT, T - j0)
                    G = Gq[t]
                    if t + PRE < NBANDS:
                        Gq.append(gather_band(t + PRE))
                    # normalize gathered rows: G[rel*8+c, :] *= r[c, :]
                    nc.vector.tensor_tensor(G[:], G[:], Rg[:],
                                            mybir.AluOpType.mult)
                    pc = ps.tile([112, 2 * T], FP32, tag="pb")
                    # ki=1 full first, then ki=0 / ki=2 shifted
                    nc.tensor.matmul(pc[:], conv_stat[:, 1, :], G[:],
                                     start=True, stop=False)
                    # ki=0: out cols i in [1,T) <- G cols i-1 ; ki=2 opposite
                    for bb in range(2):
                        nc.tensor.matmul(
                            pc[:, bb * T + 1: (bb + 1) * T],
                            conv_stat[:, 0, :],
                            G[:, bb * T: (bb + 1) * T - 1],
                            start=False, stop=False)
                        nc.tensor.matmul(
                            pc[:, bb * T: (bb + 1) * T - 1],
                            conv_stat[:, 2, :],
                            G[:, bb * T + 1: (bb + 1) * T],
                            start=False,
                            stop=(bb == 1 and not has_sconv_bias))
                    if has_sconv_bias:
                        nc.tensor.matmul(pc[:], bias_row[:], onesN[:],
                                         start=False, stop=True)
                    CG = cgp.tile([112, 2 * T], BF16, tag="CG")
                    if t % 2 == 0:
                        nc.vector.tensor_copy(CG[:], pc[:])
                    else:
                        nc.scalar.copy(CG[:], pc[:])
                    # degroup: CG rows (jrel*8+o) -> AC[jt][j, (h=o, b, i)]
                    # split at the midpoint + jt crossings, rotating queues
                    dq = 2 * t + 1
                    dcuts = sorted({j0, j0 + nj} |
                                   {c for c in (j0 + nj // 2, 128)
                                    if j0 < c < j0 + nj})
                    for seg, seg_hi in zip(dcuts[:-1], dcuts[1:]):
                        jt_ = seg // 128
                        engs[dq % 3].dma_start(
                            AC[jt_][seg - jt_ * 128: seg_hi - jt_ * 128, :, :, :],
                            CG[8 * (seg - j0): 8 * (seg_hi - j0), :])
                        dq += 1

                # ============ spectral conv -> attn_sc[bb] ============
                attn_sc = []
                for bb in range(2):
                    asc = ascp.tile([128, 4, C], BF16, tag=f"asc{bb}")
                    for kt in range(4):
                        psc = ps.tile([128, C], FP32, tag="pb")
                        for qh in range(2):
                            HS = HSq[(bb, 2 * kt + qh)]
                            po = 64 * qh
                            nc.tensor.matmul(psc[po:po + 64, :], spec_stat[:, 1, :],
                                             HS[:], start=True, stop=False)
                            nc.tensor.matmul(psc[po:po + 64, 1:], spec_stat[:, 0, :],
                                             HS[:, :C - 1], start=False, stop=False)
                            nc.tensor.matmul(psc[po:po + 64, :C - 1], spec_stat[:, 2, :],
                                             HS[:, 1:], start=False, stop=True)
                        if kt % 2 == 0:
                            nc.vector.tensor_scalar(
                                asc[:, kt, :], psc[:], bsp_col[:, 0:1], None,
                                mybir.AluOpType.add)
                        else:
                            nc.scalar.activation(
                                asc[:, kt, :], psc[:],
                                mybir.ActivationFunctionType.Identity,
                                bias=bsp_col[:, 0:1])
                    attn_sc.append(asc)

                # ============ attn@v -> OF[g] [128, (b, t)] bf16 ============
                OF = []
                for g in range(4):
                    pav = ps.tile([128, 2 * T], FP32, tag="pb", name="pav")
                    for hh in range(2):
                        h = 2 * g + hh
                        for bb in range(2):
                            for jt in range(2):
                                nc.tensor.matmul(
                                    pav[64 * hh:64 * hh + 64, bb * T:(bb + 1) * T],
                                    v_sb[bb][jt][:, h * D:(h + 1) * D],
                                    AC[jt][:, h, bb, :],
                                    start=(jt == 0), stop=(jt == 1))
                    t_ = qkp.tile([128, 2 * T], BF16, tag=f"qk{g}", name="OF")
                    if g % 2 == 0:
                        nc.vector.tensor_copy(t_[:], pav[:])
                    else:
                        nc.scalar.copy(t_[:], pav[:])
                    OF.append(t_)

                # ============ proj -> outputT [c, (b, t)] bf16 ============
                outT = []
                for ct in range(4):
                    pp = ps.tile([128, 2 * T], FP32, tag="pb", name="pp")
                    for kt in range(4):
                        nc.tensor.matmul(
                            pp[:], w_out_sb[kt][:, ct * 128:(ct + 1) * 128],
                            OF[kt][:], start=(kt == 0), stop=(kt == 3))
                    t_ = qkp.tile([128, 2 * T], BF16, tag=f"qk{4 + ct}", name="oT")
                    nc.vector.tensor_scalar(
                        t_[:], pp[:], b_out_col[:, ct:ct + 1], None,
                        mybir.AluOpType.add)
                    outT.append(t_)

                # ============ final matmul + store ============
                for bb in range(2):
                    for tt2 in range(2):
                        pf = ps.tile([128, C], FP32, tag="pb", name="pf")
                        for kt in range(4):
                            nc.tensor.matmul(
                                pf[:],
                                outT[kt][:, bb * T + tt2 * 128: bb * T + (tt2 + 1) * 128],
                                attn_sc[bb][:, kt, :],
                                start=(kt == 0), stop=(kt == 3))
                        rt = resp.tile([128, C], FP32, tag="res")
                        if (bb + tt2) % 2 == 0:
                            nc.vector.tensor_copy(rt[:], pf[:])
                        else:
                            nc.scalar.copy(rt[:], pf[:])
                        (nc.sync if bb % 2 == 0 else nc.scalar).dma_start(
                            out_d[2 * pg + bb, tt2 * 128:(tt2 + 1) * 128, :], rt[:])

    nc.compile()
    return nc


def _prep_inputs(inputs):
    x = np.asarray(inputs["x"], np.float32)
    w_qkv = np.asarray(inputs["w_qkv"], np.float32)
    w_out = np.asarray(inputs["w_out"], np.float32)
    b_out = np.asarray(inputs["b_out"], np.float32)
    w_sconv = np.asarray(inputs["w_sconv"], np.float32)
    b_sconv = np.asarray(inputs["b_sconv"], np.float32)
    w_specconv = np.asarray(inputs["w_specconv"], np.float32)
    b_specconv = np.asarray(inputs["b_specconv"], np.float32)
    w_qkv_spec = np.asarray(inputs["w_qkv_spec"], np.float32)

    has_sconv_bias = bool(np.any(b_sconv != 0))

    common = {
        "w_qkv": _to_bf16(w_qkv),
        "w_out_bf": _to_bf16(w_out),
        "b_out_col": np.ascontiguousarray(b_out.reshape(4, 128).T),
        "w_spec_q": _to_bf16(w_qkv_spec[:, :T]),
        "w_spec_k": _to_bf16(w_qkv_spec[:, T:2 * T]),
        "conv_stat": _to_bf16(build_conv_stationaries(w_sconv).transpose(1, 0, 2)),
        "spec_stat": _to_bf16(build_spec_stationaries(w_specconv[0, 0]).transpose(1, 0, 2)),
        "bias_row": _to_bf16(np.tile(b_sconv, CONV_OUT)[None, :]),
        "bsp_col": np.full((128, 1), b_specconv[0], np.float32),
        "ones8": _to_bf16(_ones8()),
        "sel16": _to_bf16(_sel16()),
        "onesN": _to_bf16(np.ones((1, C))),
        "ident": _to_bf16(np.eye(128)),
        "zeros16": _to_bf16(np.zeros((128, 2 * T))),
    }
    in_maps = []
    for core in range(NCORES):
        m = dict(common)
        m["x"] = _to_bf16(np.ascontiguousarray(
            x[core * NB:(core + 1) * NB]
            .reshape(NB, 2, 128, C).transpose(0, 2, 1, 3)))
        in_maps.append(m)
    return in_maps, has_sconv_bias


def kernel(**inputs):
    in_maps, has_sconv_bias = _prep_inputs(inputs)
    key = ("v1", has_sconv_bias)
    if key not in _cache:
        _cache[key] = build_program(has_sconv_bias)
    nc = _cache[key]
    trace = bool(int(os.environ.get("KERNEL_TRACE", "0")))
    res = run_bass_kernel_spmd(nc, in_maps, list(range(NCORES)), trace=trace)
    if trace and res.exec_time_ns is not None:
        kernel.last_exec_time_ns = res.exec_time_ns
        kernel.last_profile = res
    out = np.concatenate(
        [np.asarray(res.results[i]["out"]).astype(np.float32)
         for i in range(NCORES)], axis=0)
    return out


kernel.last_exec_time_ns = None
kernel.last_profile = None



# revision 103
# speedup vs baseline: 1.0144x; 1.0144x over previous
"""Trainium2 Bass kernel for nn_Attention_12429635355261 (sparse_attention).

Data-parallel over batch: 32 batch items -> 8 NeuronCores x 4.
Per core, batch items are processed in 2 pair-groups of 2 (free dims pack
the pair side-by-side so matmul moving free = 512).

All layouts keep the matmul contraction on partitions:
  spatial:  qT/kT [(h d), t];  dotsT/exp/attn in [j, i] layout;
            conv via grouped (c, j'-band) partition tiles + banded stationaries;
            attn@v -> OF [(h d), i]; proj -> outputT [c, t]
  spectral: qsT/ksT [u, c]; dots_s [c, c'] natural; conv via 64-row halo bands
  final:    result [t, e] = outputT.T @ attn_s_conv
"""
import os
import numpy as np

import concourse.bass as bass
import concourse.mybir as mybir
from concourse import bacc, tile
from concourse.tile import add_dep_helper
from concourse.bass_utils import run_bass_kernel_spmd

FP32 = mybir.dt.float32
FP32R = mybir.dt.float32r
BF16 = mybir.dt.bfloat16

H, D = 8, 64
T = 256
C = 512
SCALE = D ** -0.5
NCORES = 8
NB = 4            # batch per core
NPAIR = 2         # pair-groups per core

CONV_OUT = 14
CONV_IN = 16
NBANDS = (T + CONV_OUT - 1) // CONV_OUT   # 19

SB_OUT = 64
SB_IN = 66
NSB = C // SB_OUT                          # 8

_cache = {}


def _to_bf16(a):
    import ml_dtypes
    return np.asarray(a, np.float32).astype(ml_dtypes.bfloat16)


def build_conv_stationaries(w_sconv):
    LH = np.zeros((3, 128, 112), np.float32)
    for ki in range(3):
        for c in range(8):
            for rel in range(16):
                for o in range(8):
                    for jrel in range(14):
                        kj = rel - jrel
                        if 0 <= kj <= 2:
                            LH[ki, rel * 8 + c, jrel * 8 + o] = \
                                w_sconv[o, c, ki, kj]
    return LH


def build_spec_stationaries(w3):
    B = np.zeros((3, SB_IN, SB_OUT), np.float32)
    for de in range(3):
        for r in range(SB_IN):
            lo = max(0, r - 2)
            hi = min(SB_OUT - 1, r)
            for m in range(lo, hi + 1):
                dc = r - m
                if 0 <= dc <= 2:
                    B[de, r, m] = w3[dc, de]
    return B


def _ones8():
    s = np.zeros((128, 8, 8), np.float32)
    for h in range(8):
        s[:, h, h] = 1.0
    return s


def _sel16():
    s = np.zeros((8, 128), np.float32)
    for c in range(8):
        for rel in range(16):
            s[c, rel * 8 + c] = 1.0
    return s


def build_program(has_sconv_bias):
    nc = bacc.Bacc("TRN2", target_bir_lowering=False, debug=False)

    def din(name, shape, dt=FP32):
        return nc.dram_tensor(name, list(shape), dt, kind="ExternalInput").ap()

    x_d = din("x", (NB, T, C), BF16)
    w_qkv_d = din("w_qkv", (C, 3 * H * D), BF16)
    w_out_d = din("w_out_bf", (C, C), BF16)
    b_out_d = din("b_out_col", (128, 4))
    w_spq_d = din("w_spec_q", (T, T), BF16)
    w_spk_d = din("w_spec_k", (T, T), BF16)
    conv_stat_d = din("conv_stat", (128, 3, 112), BF16)
    spec_stat_d = din("spec_stat", (SB_IN, 3, SB_OUT), BF16)
    bias_row_d = din("bias_row", (1, 112), BF16)       # b_sconv expanded
    bsp_col_d = din("bsp_col", (128, 1))               # b_specconv bcast
    ones8_d = din("ones8", (128, 8, 8), BF16)          # denominator selectors
    sel16_d = din("sel16", (8, 128), BF16)             # rel-broadcast selector
    onesN_d = din("onesN", (1, C), BF16)               # rhs for bias rank-1
    ident_d = din("ident", (128, 128), BF16)           # PE transpose identity
    zeros_d = din("zeros16", (128, 2 * T), BF16)       # pad rows source
    out_d = nc.dram_tensor("out", [NB, T, C], FP32, kind="ExternalOutput").ap()

    r32 = lambda ap: ap.bitcast(FP32R)

    from contextlib import ExitStack
    with ExitStack() as stk:
        tc = stk.enter_context(tile.TileContext(nc))
        pool = lambda name, bufs, **kw: stk.enter_context(
            tc.tile_pool(name=name, bufs=bufs, **kw))
        wres = pool("wres", 1)
        xin = pool("xin", 2)
        xtp = pool("xtp", 2)
        qkp = pool("qkp", 2)
        vp = pool("vp", 2)
        ep = pool("ep", 2)
        rp = pool("rp", 2)
        gp = pool("gp", 6)
        cgp = pool("cgp", 6)
        acp = pool("acp", 2)
        spqk = pool("spqk", 2)
        asp = pool("asp", 1)
        hsp = pool("hsp", 1)
        ascp = pool("ascp", 1)
        resp = pool("resp", 2)
        ps = pool("ps", 7, space="PSUM")
        pss = pool("pss", 1, space="PSUM")
        if True:
            # ---- preload x for both pair-groups (ahead of weights; each
            # tile is split in half across the three DMA-capable queues --
            # one queue moves only ~20 GB/s, a full tile is 256 KB) ----
            x_all = []
            x_engs = [nc.gpsimd, nc.sync, nc.scalar]

            def load_x(pg, xq0):
                xp = []
                xq = xq0
                for bb in range(2):
                    t_ = xin.tile([128, 2, C], BF16, tag=f"x{bb}",
                                  name=f"x_{pg}_{bb}")
                    src = x_d[2 * pg + bb].rearrange("(tt p) c -> p tt c", p=128)
                    for half in range(2):
                        x_engs[xq % 3].dma_start(
                            t_[64 * half:64 * (half + 1), :, :],
                            src[64 * half:64 * (half + 1)])
                        xq += 1
                    xp.append(t_)
                x_all.append(xp)

            # ident first (transposes gate on it); then pg0's x
            ident = wres.tile([128, 128], BF16, tag="ident")
            nc.sync.dma_start(ident[:], ident_d[:])
            load_x(0, 0)
            load_x(1, 1)
            w_spq_sb, w_spk_sb = [], []
            for tt in range(2):
                t_ = wres.tile([128, T], BF16, tag=f"wspq{tt}")
                nc.sync.dma_start(t_[:], w_spq_d[tt * 128:(tt + 1) * 128, :])
                w_spq_sb.append(t_)
                t_ = wres.tile([128, T], BF16, tag=f"wspk{tt}")
                nc.scalar.dma_start(t_[:], w_spk_d[tt * 128:(tt + 1) * 128, :])
                w_spk_sb.append(t_)
            w_qkv_sb = []
            for ct in range(4):
                t_ = wres.tile([128, 3 * H * D], BF16, tag=f"wqkv{ct}")
                (nc.sync if ct % 2 == 0 else nc.scalar).dma_start(
                    t_[:], w_qkv_d[ct * 128:(ct + 1) * 128, :])
                w_qkv_sb.append(t_)
            zeros16 = wres.tile([128, 2 * T], BF16, tag="zeros16")
            nc.sync.dma_start(zeros16[:], zeros_d[:])
            ones8 = wres.tile([128, 8, 8], BF16, tag="ones8")
            nc.sync.dma_start(ones8[:], ones8_d[:])
            w_out_sb = []
            for kt in range(4):
                t_ = wres.tile([128, C], BF16, tag=f"wout{kt}")
                (nc.sync if kt % 2 == 0 else nc.scalar).dma_start(
                    t_[:], w_out_d[kt * 128:(kt + 1) * 128, :])
                w_out_sb.append(t_)
            conv_stat = wres.tile([128, 3, 112], BF16, tag="cstat")
            nc.sync.dma_start(conv_stat[:], conv_stat_d[:])
            spec_stat = wres.tile([SB_IN, 3, SB_OUT], BF16, tag="sstat")
            nc.scalar.dma_start(spec_stat[:], spec_stat_d[:])
            b_out_col = wres.tile([128, 4], FP32, tag="bout")
            nc.sync.dma_start(b_out_col[:], b_out_d[:])
            bias_row = wres.tile([1, 112], BF16, tag="brow")
            nc.scalar.dma_start(bias_row[:], bias_row_d[:])
            bsp_col = wres.tile([128, 1], FP32, tag="bsp")
            nc.sync.dma_start(bsp_col[:], bsp_col_d[:])
            sel16 = wres.tile([8, 128], BF16, tag="sel16")
            nc.scalar.dma_start(sel16[:], sel16_d[:])
            onesN = wres.tile([1, C], BF16, tag="onesN")
            nc.sync.dma_start(onesN[:], onesN_d[:])

            # ============ xT via PE transpose ============
            # xT2[ct] [128, (bb, t)=512] f32
            def emit_transposes(pg):
                x_t = x_all[pg]
                xT2 = []
                for ct in range(4):
                    dst = xtp.tile([128, 2 * T], BF16, tag=f"xT{ct}",
                                   name=f"xT{ct}_{pg}")
                    for bb in range(2):
                        pt = ps.tile([128, T], FP32, tag="pb", name="pt")
                        for tt in range(2):
                            nc.tensor.matmul(
                                pt[:, tt * 128:(tt + 1) * 128],
                                x_t[bb][:, tt, ct * 128:(ct + 1) * 128],
                                ident[:], start=True, stop=True)
                        nc.vector.tensor_copy(dst[:, bb * T:(bb + 1) * T], pt[:])
                    xT2.append(dst)
                return xT2

            xT2_cache = {}
            for pg in range(NPAIR):
                x_t = x_all[pg]
                xT2 = xT2_cache.pop(pg, None)
                if xT2 is None:
                    xT2 = emit_transposes(pg)

                # ============ spectral qsT/ksT [u, c] bf16 ============
                # emitted before the conv so the conv gathers (scalar/sync
                # queues) execute while the PE runs the spectral matmuls
                sq = [[None] * 2 for _ in range(2)]   # [bb][ut] q
                sk = [[None] * 2 for _ in range(2)]
                for bb in range(2):
                    for ut in range(2):
                        for which, wsb, store in ((0, w_spq_sb, sq), (1, w_spk_sb, sk)):
                            pq = ps.tile([128, C], FP32, tag="pb", name="psq")
                            for tt in range(2):
                                nc.tensor.matmul(
                                    pq[:],
                                    wsb[tt][:, ut * 128:(ut + 1) * 128],
                                    x_t[bb][:, tt, :],
                                    start=(tt == 0), stop=(tt == 1))
                            t_ = spqk.tile([128, C], BF16, tag=f"sp{which}{bb}{ut}")
                            if (bb + ut) % 2 == 0:
                                nc.vector.tensor_copy(t_[:], pq[:])
                            else:
                                nc.scalar.copy(t_[:], pq[:])
                            store[bb][ut] = t_

                # ============ dots_s + softmax -> attn_s[bb] ============
                # attn_s[bb] tensor [128, (ct4, e)] bf16
                attn_s = []
                for bb in range(2):
                    asb = asp.tile([128, 4, C], BF16, tag=f"as{bb}")
                    ssum = rp.tile([128, 4], FP32, tag=f"ss{bb}")
                    for ct4 in range(4):
                        pd = ps.tile([128, C], FP32, tag="pb", name="pds")
                        for ut in range(2):
                            nc.tensor.matmul(
                                pd[:],
                                sq[bb][ut][:, ct4 * 128:(ct4 + 1) * 128],
                                sk[bb][ut][:], start=(ut == 0), stop=(ut == 1))
                        nc.scalar.activation(
                            asb[:, ct4, :], pd[:],
                            mybir.ActivationFunctionType.Exp, scale=SCALE,
                            accum_out=ssum[:, ct4:ct4 + 1])
                    rr = rp.tile([128, 4], FP32, tag=f"rs{bb}")
                    nc.vector.reciprocal(rr[:], ssum[:])
                    for ct4 in range(4):
                        if ct4 % 2 == 0:
                            nc.scalar.activation(
                                asb[:, ct4, :], asb[:, ct4, :],
                                mybir.ActivationFunctionType.Identity,
                                scale=rr[:, ct4:ct4 + 1])
                        else:
                            nc.vector.tensor_scalar(
                                asb[:, ct4, :], asb[:, ct4, :],
                                rr[:, ct4:ct4 + 1], None,
                                mybir.AluOpType.mult)
                    attn_s.append(asb)

                def gather_hs(bb, q):
                    c0 = SB_OUT * q
                    HS = hsp.tile([SB_IN, C], BF16, tag=f"HS{bb}{q}",
                                  name=f"HS{bb}{q}_{pg}")
                    # halo rows c' in [c0-1, c0+65)
                    lo = c0 - 1
                    src_lo = max(lo, 0)
                    src_hi = min(lo + SB_IN, C)
                    dma_eng = (nc.sync, nc.gpsimd, nc.scalar)[q % 3]
                    seg = src_lo
                    while seg < src_hi:
                        ct_ = seg // 128
                        seg_hi = min(src_hi, (ct_ + 1) * 128)
                        dma_eng.dma_start(
                            HS[seg - lo: seg_hi - lo, :],
                            attn_s[bb][seg - ct_ * 128: seg_hi - ct_ * 128,
                                       ct_, :])
                        seg = seg_hi
                    if q == 0:
                        dma_eng.dma_start(HS[0:1, :], zeros16[0:1, :C])
                    if lo + SB_IN > C:
                        dma_eng.dma_start(HS[SB_IN - 1:SB_IN, :],
                                          zeros16[0:1, :C])
                    return HS

                HSq = {(bb, q): gather_hs(bb, q)
                       for bb in range(2) for q in range(8)}

                # ============ qkT: [(h d), (b, t)] bf16, 8 M-tiles ============
                qkT = []
                for mt in range(8):
                    pq = ps.tile([128, 2 * T], FP32, tag="pb")
                    for ct in range(4):
                        nc.tensor.matmul(
                            pq[:], w_qkv_sb[ct][:, mt * 128:(mt + 1) * 128],
                            xT2[ct][:], start=(ct == 0), stop=(ct == 3))
                    t_ = qkp.tile([128, 2 * T], BF16, tag=f"qk{mt}")
                    if mt % 2 == 0:
                        nc.vector.tensor_copy(t_[:], pq[:])
                    else:
                        nc.scalar.copy(t_[:], pq[:])
                    qkT.append(t_)

                # ============ v: [t, (h d)] bf16 per (bb, tt) ============
                v_sb = [[None] * 2 for _ in range(2)]
                for bb in range(2):
                    for tt in range(2):
                        pv = ps.tile([128, C], FP32, tag="pb")
                        for ct in range(4):
                            nc.tensor.matmul(
                                pv[:],
                                xT2[ct][:, bb * T + tt * 128: bb * T + (tt + 1) * 128],
                                w_qkv_sb[ct][:, 2 * H * D:],
                                start=(ct == 0), stop=(ct == 3))
                        t_ = vp.tile([128, C], BF16, tag=f"v{bb}{tt}")
                        if (bb + tt) % 2 == 0:
                            nc.vector.tensor_copy(t_[:], pv[:])
                        else:
                            nc.scalar.copy(t_[:], pv[:])
                        v_sb[bb][tt] = t_

                # ============ dotsT + exp -> E [128, (h, jt, b, i)] bf16 ====
                E = ep.tile([128, H, 2, 2, T], BF16, tag="E")
                # jt-outer: all jt=0 exps land first so the pre-issued conv
                # gathers for bands 0..8 (jt=0 rows only) start mid-dots
                for jt in range(2):
                    for h in range(H):
                        hp = 64 * (h % 2)
                        pd = ps.tile([128, 2 * T], FP32, tag="pb")
                        for bb in range(2):
                            nc.tensor.matmul(
                                pd[:, bb * T:(bb + 1) * T],
                                qkT[4 + h // 2][hp:hp + 64,
                                                bb * T + jt * 128: bb * T + (jt + 1) * 128],
                                qkT[h // 2][hp:hp + 64, bb * T:(bb + 1) * T],
                                start=True, stop=True)
                        # exp(scale * dots) -> E block  [128, (b, i)]
                        nc.scalar.activation(
                            E[:, h, jt, :, :], pd[:],
                            mybir.ActivationFunctionType.Exp, scale=SCALE)

                # ============ softmax denominators (batched) ============
                # sp16[h] = sum_j E[j, h, :, :, :] ; rr16 = 1/sp16 ;
                # Rg[rel*8+c, :] = rr16[c, :]  (grouped broadcast for G scale)
                sp16 = pss.tile([8, 2 * T], FP32, tag="s")
                for h in range(H):
                    for jt in range(2):
                        nc.tensor.matmul(
                            sp16[:], ones8[:, h, :], E[:, h, jt, :, :],
                            start=(h == 0 and jt == 0),
                            stop=(h == H - 1 and jt == 1))
                rr16 = rp.tile([8, 2 * T], BF16, tag="rr")
                with nc.allow_low_precision(reason="bf16 recip"):
                    nc.vector.reciprocal(rr16[:], sp16[:])
                Rg_ps = ps.tile([128, 2 * T], FP32, tag="pb", name="Rg_ps")
                nc.tensor.matmul(Rg_ps[:], sel16[:], rr16[:],
                                 start=True, stop=True)
                Rg = rp.tile([128, 2 * T], BF16, tag="Rg")
                nc.vector.tensor_copy(Rg[:], Rg_ps[:])

                # ---- conv gather stage (reads RAW E -- independent of the
                # softmax denominators; pre-issued so transfers overlap the
                # spectral matmuls below) ----
                engs = (nc.scalar, nc.sync, nc.gpsimd)

                def gather_band(t):
                    j0 = CONV_OUT * t
                    G = gp.tile([128, 2 * T], BF16, tag="G", name=f"G{t}_{pg}")
                    lo = j0 - 1
                    src_lo = max(lo, 0)
                    src_hi = min(lo + CONV_IN, T)
                    gq = 2 * t
                    cuts = sorted({src_lo, src_hi} |
                                  {c for c in (lo + 8, 128)
                                   if src_lo < c < src_hi})
                    for seg, seg_hi in zip(cuts[:-1], cuts[1:]):
                        jt_ = seg // 128
                        p0 = seg - lo
                        engs[gq % 3].dma_start(
                            G[8 * p0: 8 * (p0 + seg_hi - seg), :],
                            E[seg - jt_ * 128: seg_hi - jt_ * 128, :, jt_, :, :])
                        gq += 1
                    if t == 0:
                        engs[gq % 3].dma_start(G[0:8, :], zeros16[0:8, :])
                        gq += 1
                    if j0 - 1 + CONV_IN > T:
                        npad = j0 - 1 + CONV_IN - T
                        engs[gq % 3].dma_start(G[8 * (CONV_IN - npad):128, :],
                                               zeros16[:8 * npad, :])
                    return G

                PRE = 5
                Gq = [gather_band(t) for t in range(PRE)]

                # ============ spatial conv (grouped bands) ============
                # AC[jt] [128, (h, b, i)] bf16
                AC = [acp.tile([128, H, 2, T], BF16, tag=f"AC{jt}", name=f"AC{jt}_{pg}")
                      for jt in range(2)]
                for t in range(NBANDS):
                    j0 = CONV_OUT * t
                    nj = min(CONV_OUT, T - j0)
                    G = Gq[t]
                    if t + PRE < NBANDS:
                        Gq.append(gather_band(t + PRE))
                    # normalize gathered rows: G[rel*8+c, :] *= r[c, :]
                    nc.vector.tensor_tensor(G[:], G[:], Rg[:],
                                            mybir.AluOpType.mult)
                    pc = ps.tile([112, 2 * T], FP32, tag="pb")
                    # ki=1 full first, then ki=0 / ki=2 shifted
                    nc.tensor.matmul(pc[:], conv_stat[:, 1, :], G[:],
                                     start=True, stop=False)
                    # ki=0: out cols i in [1,T) <- G cols i-1 ; ki=2 opposite
                    for bb in range(2):
                        nc.tensor.matmul(
                            pc[:, bb * T + 1: (bb + 1) * T],
                            conv_stat[:, 0, :],
                            G[:, bb * T: (bb + 1) * T - 1],
                            start=False, stop=False)
                        nc.tensor.matmul(
                            pc[:, bb * T: (bb + 1) * T - 1],
                            conv_stat[:, 2, :],
                            G[:, bb * T + 1: (bb + 1) * T],
                            start=False,
                            stop=(bb == 1 and not has_sconv_bias))
                    if has_sconv_bias:
                        nc.tensor.matmul(pc[:], bias_row[:], onesN[:],
                                         start=False, stop=True)
                    CG = cgp.tile([112, 2 * T], BF16, tag="CG")
                    if t % 2 == 0:
                        nc.vector.tensor_copy(CG[:], pc[:])
                    else:
                        nc.scalar.copy(CG[:], pc[:])
                    # degroup: CG rows (jrel*8+o) -> AC[jt][j, (h=o, b, i)]
                    # split at the midpoint + jt crossings, rotating queues
                    dq = 2 * t + 1
                    dcuts = sorted({j0, j0 + nj} |
                                   {c for c in (j0 + nj // 2, 128)
                                    if j0 < c < j0 + nj})
                    for seg, seg_hi in zip(dcuts[:-1], dcuts[1:]):
                        jt_ = seg // 128
                        engs[dq % 3].dma_start(
                            AC[jt_][seg - jt_ * 128: seg_hi - jt_ * 128, :, :, :],
                            CG[8 * (seg - j0): 8 * (seg_hi - j0), :])
                        dq += 1

                # ============ spectral conv -> attn_sc[bb] ============
                attn_sc = []
                for bb in range(2):
                    asc = ascp.tile([128, 4, C], BF16, tag=f"asc{bb}")
                    for kt in range(4):
                        psc = ps.tile([128, C], FP32, tag="pb")
                        for qh in range(2):
                            HS = HSq[(bb, 2 * kt + qh)]
                            po = 64 * qh
                            nc.tensor.matmul(psc[po:po + 64, :], spec_stat[:, 1, :],
                                             HS[:], start=True, stop=False)
                            nc.tensor.matmul(psc[po:po + 64, 1:], spec_stat[:, 0, :],
                                             HS[:, :C - 1], start=False, stop=False)
                            nc.tensor.matmul(psc[po:po + 64, :C - 1], spec_stat[:, 2, :],
                                             HS[:, 1:], start=False, stop=True)
                        if kt % 2 == 0:
                            nc.vector.tensor_scalar(
                                asc[:, kt, :], psc[:], bsp_col[:, 0:1], None,
                                mybir.AluOpType.add)
                        else:
                            nc.scalar.activation(
                                asc[:, kt, :], psc[:],
                                mybir.ActivationFunctionType.Identity,
                                bias=bsp_col[:, 0:1])
                    attn_sc.append(asc)

                # ============ attn@v -> OF[g] [128, (b, t)] bf16 ============
                OF = []
                for g in range(4):
                    pav = ps.tile([128, 2 * T], FP32, tag="pb", name="pav")
                    for hh in range(2):
                        h = 2 * g + hh
                        for bb in range(2):
                            for jt in range(2):
                                nc.tensor.matmul(
                                    pav[64 * hh:64 * hh + 64, bb * T:(bb + 1) * T],
                                    v_sb[bb][jt][:, h * D:(h + 1) * D],
                                    AC[jt][:, h, bb, :],
                                    start=(jt == 0), stop=(jt == 1))
                    t_ = qkp.tile([128, 2 * T], BF16, tag=f"qk{g}", name="OF")
                    if g % 2 == 0:
                        nc.vector.tensor_copy(t_[:], pav[:])
                    else:
                        nc.scalar.copy(t_[:], pav[:])
                    OF.append(t_)

                # ============ proj -> outputT [c, (b, t)] bf16 ============
                outT = []
                for ct in range(4):
                    pp = ps.tile([128, 2 * T], FP32, tag="pb", name="pp")
                    for kt in range(4):
                        nc.tensor.matmul(
                            pp[:], w_out_sb[kt][:, ct * 128:(ct + 1) * 128],
                            OF[kt][:], start=(kt == 0), stop=(kt == 3))
                    t_ = qkp.tile([128, 2 * T], BF16, tag=f"qk{4 + ct}", name="oT")
                    nc.vector.tensor_scalar(
                        t_[:], pp[:], b_out_col[:, ct:ct + 1], None,
                        mybir.AluOpType.add)
                    outT.append(t_)

                # ============ final matmul + store ============
                for bb in range(2):
                    for tt2 in range(2):
                        pf = ps.tile([128, C], FP32, tag="pb", name="pf")
                        for kt in range(4):
                            nc.tensor.matmul(
                                pf[:],
                                outT[kt][:, bb * T + tt2 * 128: bb * T + (tt2 + 1) * 128],
                                attn_sc[bb][:, kt, :],
                                start=(kt == 0), stop=(kt == 3))
                        rt = resp.tile([128, C], FP32, tag="res")
                        if (bb + tt2) % 2 == 0:
                            nc.vector.tensor_copy(rt[:], pf[:])
                        else:
                            nc.scalar.copy(rt[:], pf[:])
                        (nc.sync if bb % 2 == 0 else nc.scalar).dma_start(
                            out_d[2 * pg + bb, tt2 * 128:(tt2 + 1) * 128, :], rt[:])

    nc.compile()
    return nc


def _prep_inputs(inputs):
    x = np.asarray(inputs["x"], np.float32)
    w_qkv = np.asarray(inputs["w_qkv"], np.float32)
    w_out = np.asarray(inputs["w_out"], np.float32)
    b_out = np.asarray(inputs["b_out"], np.float32)
    w_sconv = np.asarray(inputs["w_sconv"], np.float32)
    b_sconv = np.asarray(inputs["b_sconv"], np.float32)
    w_specconv = np.asarray(inputs["w_specconv"], np.float32)
    b_specconv = np.asarray(inputs["b_specconv"], np.float32)
    w_qkv_spec = np.asarray(inputs["w_qkv_spec"], np.float32)

    has_sconv_bias = bool(np.any(b_sconv != 0))

    common = {
        "w_qkv": _to_bf16(w_qkv),
        "w_out_bf": _to_bf16(w_out),
        "b_out_col": np.ascontiguousarray(b_out.reshape(4, 128).T),
        "w_spec_q": _to_bf16(w_qkv_spec[:, :T]),
        "w_spec_k": _to_bf16(w_qkv_spec[:, T:2 * T]),
        "conv_stat": _to_bf16(build_conv_stationaries(w_sconv).transpose(1, 0, 2)),
        "spec_stat": _to_bf16(build_spec_stationaries(w_specconv[0, 0]).transpose(1, 0, 2)),
        "bias_row": _to_bf16(np.tile(b_sconv, CONV_OUT)[None, :]),
        "bsp_col": np.full((128, 1), b_specconv[0], np.float32),
        "ones8": _to_bf16(_ones8()),
        "sel16": _to_bf16(_sel16()),
        "onesN": _to_bf16(np.ones((1, C))),
        "ident": _to_bf16(np.eye(128)),
        "zeros16": _to_bf16(np.zeros((128, 2 * T))),
    }
    in_maps = []
    for core in range(NCORES):
        m = dict(common)
        m["x"] = _to_bf16(x[core * NB:(core + 1) * NB])
        in_maps.append(m)
    return in_maps, has_sconv_bias


def kernel(**inputs):
    in_maps, has_sconv_bias = _prep_inputs(inputs)
    key = ("v1", has_sconv_bias)
    if key not in _cache:
        _cache[key] = build_program(has_sconv_bias)
    nc = _cache[key]
    trace = bool(int(os.environ.get("KERNEL_TRACE", "0")))
    res = run_bass_kernel_spmd(nc, in_maps, list(range(NCORES)), trace=trace)
    if trace and res.exec_time_ns is not None:
        kernel.last_exec_time_ns = res.exec_time_ns
        kernel.last_profile = res
    out = np.concatenate(
        [np.asarray(res.results[i]["out"]).astype(np.float32)
         for i in range(NCORES)], axis=0)
    return out


kernel.last_exec_time_ns = None
kernel.last_profile = None

